# revision 3
# baseline (speedup 1.0000x reference)
"""Trainium2 Bass kernel for the segment_reduce loss (nn_Loss_65996467471179).

Strategy (data-parallel over curves, fp16 streaming):
  - C=65536 curves of L=256 points. Shard curves across 8 cores (8192 each).
  - The five big N-length arrays (An, A_r, Ac, Aj, Ap) are downcast to fp16
    on the host inside kernel(); each core streams its 20MB fp16 shard once
    from HBM in [128, F] chunks (F/L curves per partition row), computes all
    per-curve and global partial reductions on-chip in fp32 accumulators,
    and writes a small [128, ACCW] float32 accumulator block back to DRAM.
  - fp16 quantization of the inputs perturbs the loss by ~5e-6 relative
    (dominant term is sum relu(-Ap) ~ 6.7e6; argmin tie-flips are unbiased),
    far inside the 2e-2 gate.
  - Ci is only read at end-of-curve indices; that gather plus all C-length /
    O(4)-length pure-input terms (correlation moments, Rd25/dHa/Topt sign
    penalties) are folded on the host, which also combines the 8 cores'
    partial blocks into the final scalar in float64.

Engine assignment per chunk (per-curve rows along the free axis, J per
partition):
  GPSIMD: d = An - A_r; end-of-curve strided extracts (f16->f32)
  ACT:    Square(d) accum -> MSE col; Relu(-Ap) accum -> apn col
  DVE:    J sliced tensor_tensor_reduce: Acj = Ac - Aj with per-curve
            accum -> sAcj (fp16 2x mode)
          J sliced tensor_scalar abs_max(Acj, 0) -> A with per-curve
            accum -> sAbs (fp16 4x mode)
          2-stage pairwise min tree + 3D tensor_reduce -> mn per curve
          G = 1.1*Aj - Ap (stt)
          J sliced stt (A==mn)*G accum -> gint per curve
Modeled per-core busy: DVE ~53us, ACT ~33us, GPSIMD ~35us, DMA 20MB at
~360-440GB/s ~ 45-56us -> memory-bound.
"""

import os
import sys

import numpy as np

sys.path.insert(0, "/opt/trn_rl_repo")

import concourse.bass as bass
import concourse.bacc as bacc
import concourse.tile as tile
from concourse import mybir
from concourse.bass_utils import run_bass_kernel_spmd
from contextlib import ExitStack

NCORES = 8
C = 65536
L = 256
N = C * L
S = C // NCORES          # curves per core
NSH = S * L              # elements per core per big array
P = 128                  # partitions

KELVIN = 273.15
FIT_AP_CI = 500.0
TARGET_R = 0.7

f16 = mybir.dt.float16
f32 = mybir.dt.float32


VARIANT = dict(
    F=4096,             # elements per partition per chunk
    inp_bufs=2,
    wrk_bufs=2,
    d_on_pool=True,     # An-A_r subtract on GPSIMD (else DVE)
    apn_on_act=True,    # sum relu(-Ap) on ACT (else DVE tensor_scalar)
    mse_on_act=True,    # d^2 sum on ACT Square (always)
    ttr_acj=True,       # fused sliced TTR for Acj+sAcj (else TT + sliced ts)
    tree_stages=2,      # pairwise TT-min stages before the 3D min reduce
    ends_on_pool=True,  # end-of-curve extracts on GPSIMD (else DVE)
    epi_on_pool=True,   # epilogue tensor_tensor ops on GPSIMD (else DVE)
    dma_acj_first=True,
)


def _geom(v):
    F = v["F"]
    J = F // L
    M = NSH // (P * F)
    NCOL = M * J         # 64 for any F
    MSE0 = 0
    APN0 = MSE0 + M
    P30 = APN0 + M
    LS0 = P30 + NCOL
    E10 = LS0 + NCOL
    E20 = E10 + NCOL
    ACCW = E20 + NCOL
    return F, J, M, NCOL, MSE0, APN0, P30, LS0, E10, E20, ACCW


def _build_kernel(reps=None, variant=None):
    """reps=None: normal single-pass kernel. reps=R: wrap the whole body in a
    runtime For_i loop executing it R times (for HW timing via slope)."""
    v = dict(VARIANT)
    if variant:
        v.update(variant)
    F, J, M, NCOL, MSE0, APN0, P30, LS0, E10, E20, ACCW = _geom(v)

    nc = bacc.Bacc("TRN2", target_bir_lowering=False, debug=False, num_devices=NCORES)
    big = {
        nm: nc.declare_dram_parameter(nm, [NSH], f16, isOutput=False)
        for nm in ("An", "Ar", "Ac", "Aj", "Ap")
    }
    wdev = nc.declare_dram_parameter("wdev", [P, NCOL], f32, isOutput=False)
    fitw = nc.declare_dram_parameter("fitw", [P, NCOL], f32, isOutput=False)
    acc = nc.declare_dram_parameter("acc", [P, ACCW], f32, isOutput=True)

    with ExitStack() as ctx:
        tc = ctx.enter_context(tile.TileContext(nc))
        inp = ctx.enter_context(tc.tile_pool(name="inp", bufs=v["inp_bufs"]))
        wrk = ctx.enter_context(tc.tile_pool(name="wrk", bufs=v["wrk_bufs"]))
        per = ctx.enter_context(tc.tile_pool(name="per", bufs=1))

        g = {}
        for nm, shp, dt in (
            ("accT", [P, ACCW], f32), ("mnB", [P, NCOL], f16),
            ("sAcj", [P, NCOL], f32), ("sAbs", [P, NCOL], f32),
            ("gint", [P, NCOL], f32), ("eAp", [P, NCOL], f32),
            ("eAj", [P, NCOL], f32), ("eAc", [P, NCOL], f32),
            ("wT", [P, NCOL], f32), ("fT", [P, NCOL], f32),
            ("t1", [P, NCOL], f32), ("t2", [P, NCOL], f32),
            ("r1", [P, NCOL], f32), ("r2", [P, NCOL], f32),
            ("b8", [P, 1], f32),
        ):
            g[nm] = per.tile(shp, dt, tag=nm, name=nm)
        nc.vector.memset(g["b8"], 8.0)
        nc.sync.dma_start(out=g["wT"], in_=wdev[:])
        nc.sync.dma_start(out=g["fT"], in_=fitw[:])

        def body():
            _trace_body(nc, tc, big, acc, inp, wrk, g, v)

        if reps is None:
            body()
        else:
            with tc.For_i(0, reps, 1):
                body()

    nc.compile()
    return nc


def _trace_body(nc, tc, big, acc, inp, wrk, g, v):
    OP = mybir.AluOpType
    AF = mybir.ActivationFunctionType
    AX = mybir.AxisListType
    F, J, M, NCOL, MSE0, APN0, P30, LS0, E10, E20, ACCW = _geom(v)
    accT = g["accT"]

    for m in range(M):
        t = {}
        dma_order = ("Ac", "Aj", "Ap", "An", "Ar") if v["dma_acj_first"] \
            else ("An", "Ar", "Ac", "Aj", "Ap")
        for nm in dma_order:
            t[nm] = inp.tile([P, F], f16, tag=nm, name=f"in_{nm}_{m}")
            src = big[nm][:].rearrange("(m p f) -> m p f", m=M, p=P, f=F)[m]
            nc.sync.dma_start(out=t[nm], in_=src)

        cols = slice(m * J, (m + 1) * J)

        # --- GPSIMD: An-Ar subtract + end-of-curve extracts ---
        d = wrk.tile([P, F], f16, tag="d")
        d_eng = nc.gpsimd if v["d_on_pool"] else nc.vector
        d_eng.tensor_tensor(out=d, in0=t["An"], in1=t["Ar"], op=OP.subtract)
        ends_eng = nc.gpsimd if v["ends_on_pool"] else nc.vector
        for nm, dst in (("Ap", g["eAp"]), ("Aj", g["eAj"]), ("Ac", g["eAc"])):
            ends = t[nm].rearrange("p (j l) -> p j l", l=L)[:, :, L - 1 : L]
            ends_eng.tensor_copy(out=dst[:, cols], in_=ends)

        # --- ACT: global accumulations ---
        junk1 = wrk.tile([P, F], f16, tag="junk1")
        nc.scalar.activation(
            out=junk1, in_=d, func=AF.Square,
            accum_out=accT[:, MSE0 + m : MSE0 + m + 1],
        )
        apn_dst = accT[:, APN0 + m : APN0 + m + 1]
        if v["apn_on_act"]:
            junk2 = wrk.tile([P, F], f16, tag="junk2")
            nc.scalar.activation(
                out=junk2, in_=t["Ap"], func=AF.Relu, scale=-1.0,
                accum_out=apn_dst,
            )
        else:
            # accum = sum(min(Ap, 0)) = -sum(relu(-Ap)); negated on host.
            junk2 = wrk.tile([P, F], f16, tag="junk2")
            nc.vector.tensor_scalar(
                out=junk2, in0=t["Ap"], scalar1=0.0, scalar2=None,
                op0=OP.min, op1=OP.add, accum_out=apn_dst,
            )

        # --- DVE: Acj + per-curve sAcj (fused sliced TTR) ---
        Acj = wrk.tile([P, F], f16, tag="Acj")
        if v["ttr_acj"]:
            for j in range(J):
                c = m * J + j
                sl = slice(j * L, (j + 1) * L)
                nc.vector.tensor_tensor_reduce(
                    out=Acj[:, sl], in0=t["Ac"][:, sl], in1=t["Aj"][:, sl],
                    scale=1.0, scalar=0.0, op0=OP.subtract, op1=OP.add,
                    accum_out=g["sAcj"][:, c : c + 1],
                )
        else:
            nc.vector.tensor_tensor(out=Acj, in0=t["Ac"], in1=t["Aj"],
                                    op=OP.subtract)
            for j in range(J):
                c = m * J + j
                sl = slice(j * L, (j + 1) * L)
                junk3 = wrk.tile([P, L], f16, tag="junk3")
                nc.vector.tensor_scalar(
                    out=junk3, in0=Acj[:, sl], scalar1=1.0, scalar2=None,
                    op0=OP.mult, op1=OP.add,
                    accum_out=g["sAcj"][:, c : c + 1],
                )

        # --- DVE: A = |Acj| + per-curve sAbs (sliced ts abs_max, 4x) ---
        A = wrk.tile([P, F], f16, tag="A")
        for j in range(J):
            c = m * J + j
            sl = slice(j * L, (j + 1) * L)
            nc.vector.tensor_scalar(
                out=A[:, sl], in0=Acj[:, sl], scalar1=0.0, scalar2=None,
                op0=OP.abs_max, op1=OP.add,
                accum_out=g["sAbs"][:, c : c + 1],
            )

        # --- DVE: per-curve min via pairwise TT-min tree + 3D reduce ---
        A3 = A.rearrange("p (j l) -> p j l", l=L)
        stages = v["tree_stages"]
        cur3, ln = A3, L
        for s in range(stages):
            half = ln // 2
            Ts = wrk.tile([P, J * half], f16, tag=f"T{s}")
            Ts3 = Ts.rearrange("p (j h) -> p j h", h=half)
            nc.vector.tensor_tensor(
                out=Ts3, in0=cur3[:, :, 0:half], in1=cur3[:, :, half:ln],
                op=OP.min,
            )
            cur3, ln = Ts3, half
        nc.vector.tensor_reduce(out=g["mnB"][:, cols], in_=cur3, axis=AX.X,
                                op=OP.min)

        # --- DVE: G = 1.1*Aj - Ap; gint = sum (A==mn)*G per curve ---
        G = wrk.tile([P, F], f16, tag="G")
        nc.vector.scalar_tensor_tensor(
            out=G, in0=t["Aj"], scalar=1.1, in1=t["Ap"],
            op0=OP.mult, op1=OP.subtract,
        )
        junkD = wrk.tile([P, L], f16, tag="junkD")
        for j in range(J):
            c = m * J + j
            sl = slice(j * L, (j + 1) * L)
            nc.vector.scalar_tensor_tensor(
                out=junkD, in0=A[:, sl], scalar=g["mnB"][:, c : c + 1],
                in1=G[:, sl], op0=OP.is_equal, op1=OP.mult,
                accum_out=g["gint"][:, c : c + 1],
            )

    # --- epilogue on [128, NCOL] column blocks ---
    AF = mybir.ActivationFunctionType
    OP = mybir.AluOpType
    epi = nc.gpsimd if v["epi_on_pool"] else nc.vector
    t1, t2, r1, r2 = g["t1"], g["t2"], g["r1"], g["r2"]
    sAbs, sAcj, b8 = g["sAbs"], g["sAcj"], g["b8"]
    # ls penalty: relu(8-ls_Aj)+relu(8-ls_Ac), ls_* = (sAbs -+ sAcj)/2
    epi.tensor_tensor(out=t1, in0=sAbs, in1=sAcj, op=OP.add)
    nc.scalar.activation(out=r1, in_=t1, func=AF.Relu, scale=-0.5, bias=b8)
    epi.tensor_tensor(out=t2, in0=sAbs, in1=sAcj, op=OP.subtract)
    nc.scalar.activation(out=r2, in_=t2, func=AF.Relu, scale=-0.5, bias=b8)
    epi.tensor_tensor(out=t1, in0=r1, in1=r2, op=OP.add)
    epi.tensor_tensor(out=accT[:, LS0 : LS0 + NCOL], in0=t1, in1=g["wT"],
                      op=OP.mult)
    # crossover penalty: 3*relu(gint) == relu(3*gint)
    nc.scalar.activation(out=accT[:, P30 : P30 + NCOL], in_=g["gint"],
                         func=AF.Relu, scale=3.0)
    # end-of-curve penalties
    epi.tensor_tensor(out=t2, in0=g["eAp"], in1=g["eAj"], op=OP.subtract)
    nc.scalar.activation(out=r1, in_=t2, func=AF.Relu)
    epi.tensor_tensor(out=accT[:, E10 : E10 + NCOL], in0=r1, in1=g["fT"],
                      op=OP.mult)
    epi.tensor_tensor(out=t2, in0=g["eAj"], in1=g["eAc"], op=OP.subtract)
    nc.scalar.activation(out=accT[:, E20 : E20 + NCOL], in_=t2, func=AF.Relu)

    nc.sync.dma_start(out=acc[:], in_=accT)


_NC_CACHE = {}
LAST_RESULTS = None


def _get_nc(reps=None, variant=None):
    key = (reps, tuple(sorted((variant or {}).items())))
    if key not in _NC_CACHE:
        _NC_CACHE[key] = _build_kernel(reps, variant)
    return _NC_CACHE[key]


def _curve_layout(x_per_curve: np.ndarray, v=None) -> np.ndarray:
    """Map a per-curve [S] array for one core into the device [P, NCOL] layout:
    dev[p, m*J + j] corresponds to curve m*(P*J) + p*J + j."""
    F, J, M, NCOL = _geom(v or VARIANT)[:4]
    return np.ascontiguousarray(
        x_per_curve.reshape(M, P, J).transpose(1, 0, 2).reshape(P, NCOL)
    )


def prep_in_maps(An_o, Ac_o, Aj_o, Ap_o, A_r, Ci, mask_lightresp, v=None):
    w_full = (mask_lightresp == 0).astype(np.float32)        # [C]
    Ci_end = np.ascontiguousarray(Ci[L - 1 :: L])            # [C]
    fit_full = ((Ci_end > FIT_AP_CI).astype(np.float32) * w_full)  # [C]

    h = lambda x: np.ascontiguousarray(x, dtype=np.float16)
    in_maps = []
    for k in range(NCORES):
        cur = slice(k * S, (k + 1) * S)
        el = slice(k * NSH, (k + 1) * NSH)
        in_maps.append({
            "An": h(An_o[el]),
            "Ar": h(A_r[el]),
            "Ac": h(Ac_o[el]),
            "Aj": h(Aj_o[el]),
            "Ap": h(Ap_o[el]),
            "wdev": _curve_layout(w_full[cur], v),
            "fitw": _curve_layout(fit_full[cur], v),
        })
    return in_maps


def kernel(An_o, Ac_o, Aj_o, Ap_o, A_r, Ci, Vcmax25, Jmax25, Rd25,
           dHa_Vcmax, dHa_Jmax, dHa_TPU, Topt_Vcmax, Topt_Jmax, Topt_TPU,
           mask_lightresp):
    An_o, Ac_o, Aj_o, Ap_o, A_r, Ci = (
        np.asarray(x) for x in (An_o, Ac_o, Aj_o, Ap_o, A_r, Ci))
    (Vcmax25, Jmax25, Rd25, dHa_Vcmax, dHa_Jmax, dHa_TPU,
     Topt_Vcmax, Topt_Jmax, Topt_TPU, mask_lightresp) = (
        np.asarray(x) for x in (Vcmax25, Jmax25, Rd25, dHa_Vcmax, dHa_Jmax,
                                dHa_TPU, Topt_Vcmax, Topt_Jmax, Topt_TPU,
                                mask_lightresp))
    v = dict(VARIANT)
    F, J, M, NCOL, MSE0, APN0, P30, LS0, E10, E20, ACCW = _geom(v)
    nc = _get_nc()
    in_maps = prep_in_maps(An_o, Ac_o, Aj_o, Ap_o, A_r, Ci, mask_lightresp, v)

    try:
        res = run_bass_kernel_spmd(
            nc, in_maps, core_ids=list(range(NCORES)),
            trace=bool(int(os.environ.get("KERNEL_TRACE", "0"))),
        )
    except ModuleNotFoundError:
        os.environ["BASS_NEVER_TRACE"] = "1"
        res = run_bass_kernel_spmd(nc, in_maps, core_ids=list(range(NCORES)))
    global LAST_RESULTS
    LAST_RESULTS = res
    blocks = [r["acc"].astype(np.float64) for r in res.results]

    mse = sum(b[:, MSE0 : MSE0 + M].sum() for b in blocks)
    apn = sum(b[:, APN0 : APN0 + M].sum() for b in blocks)
    p3 = sum(b[:, P30 : P30 + NCOL].sum() for b in blocks)
    ls = sum(b[:, LS0 : LS0 + NCOL].sum() for b in blocks)
    e1 = sum(b[:, E10 : E10 + NCOL].sum() for b in blocks)
    e2 = sum(b[:, E20 : E20 + NCOL].sum() for b in blocks)
    if not v["apn_on_act"]:
        apn = -apn

    # host-side terms (tiny inputs only)
    w = (mask_lightresp == 0).astype(np.float64)
    x = Jmax25.astype(np.float64)
    y = Vcmax25.astype(np.float64)
    nw = w.sum()
    if nw > 0:
        my = (w * y).sum() / nw
        mx = (w * x).sum() / nw
        vy = (y - my) * w
        vx = (x - mx) * w
        denom = np.sqrt((vx * vx).sum()) * np.sqrt((vy * vy).sum())
        cost = (vx * vy).sum() / denom if denom != 0.0 else np.nan
    else:
        cost = np.nan
    if np.isnan(cost):
        cost = 0.0
    cost = min(cost, TARGET_R)

    relu = lambda z: np.maximum(z, 0.0)
    loss = mse * 10.0 / N
    loss += TARGET_R - cost
    loss += relu(-Rd25.astype(np.float64)).sum()
    loss += relu(-dHa_Vcmax.astype(np.float64)).sum() * 10.0
    loss += relu(-dHa_Jmax.astype(np.float64)).sum()
    loss += relu(-dHa_TPU.astype(np.float64)).sum()
    loss += relu(KELVIN - Topt_Vcmax.astype(np.float64)).sum()
    loss += relu(KELVIN - Topt_Jmax.astype(np.float64)).sum()
    loss += relu(KELVIN - Topt_TPU.astype(np.float64)).sum()
    loss += apn
    loss += e1 * 0.15
    loss += e2
    loss += p3
    loss += ls

    return np.asarray(loss, dtype=np.float32)


# revision 4
# speedup vs baseline: 1.7167x; 1.7167x over previous
"""Trainium2 Bass kernel for the segment_reduce loss (nn_Loss_65996467471179).

Strategy (data-parallel over curves, fp16 streaming):
  - C=65536 curves of L=256 points. Shard curves across 8 cores (8192 each).
  - The five big N-length arrays (An, A_r, Ac, Aj, Ap) are downcast to fp16
    on the host inside kernel(); each core streams its 20MB fp16 shard once
    from HBM in [128, F] chunks (F/L curves per partition row), computes all
    per-curve and global partial reductions on-chip in fp32 accumulators,
    and writes a small [128, ACCW] float32 accumulator block back to DRAM.
  - fp16 quantization of the inputs perturbs the loss by ~5e-6 relative
    (dominant term is sum relu(-Ap) ~ 6.7e6; argmin tie-flips are unbiased),
    far inside the 2e-2 gate.
  - Ci is only read at end-of-curve indices; that gather plus all C-length /
    O(4)-length pure-input terms (correlation moments, Rd25/dHa/Topt sign
    penalties) are folded on the host, which also combines the 8 cores'
    partial blocks into the final scalar in float64.

Engine assignment per chunk (per-curve rows along the free axis, J per
partition):
  GPSIMD: d = An - A_r; end-of-curve strided extracts (f16->f32)
  ACT:    Square(d) accum -> MSE col; Relu(-Ap) accum -> apn col
  DVE:    J sliced tensor_tensor_reduce: Acj = Ac - Aj with per-curve
            accum -> sAcj (fp16 2x mode)
          J sliced tensor_scalar abs_max(Acj, 0) -> A with per-curve
            accum -> sAbs (fp16 4x mode)
          2-stage pairwise min tree + 3D tensor_reduce -> mn per curve
          G = 1.1*Aj - Ap (stt)
          J sliced stt (A==mn)*G accum -> gint per curve
Modeled per-core busy: DVE ~53us, ACT ~33us, GPSIMD ~35us, DMA 20MB at
~360-440GB/s ~ 45-56us -> memory-bound.
"""

import os
import sys

import numpy as np

sys.path.insert(0, "/opt/trn_rl_repo")

import concourse.bass as bass
import concourse.bacc as bacc
import concourse.tile as tile
from concourse import mybir
from concourse.bass_utils import run_bass_kernel_spmd
from contextlib import ExitStack

NCORES = 8
C = 65536
L = 256
N = C * L
S = C // NCORES          # curves per core
NSH = S * L              # elements per core per big array
P = 128                  # partitions

KELVIN = 273.15
FIT_AP_CI = 500.0
TARGET_R = 0.7

f16 = mybir.dt.float16
f32 = mybir.dt.float32


VARIANT = dict(
    F=4096,             # elements per partition per chunk
    inp_bufs=2,
    wrk_bufs=2,
    d_on_pool=True,     # An-A_r subtract on GPSIMD (else DVE)
    apn_on_act=True,    # sum relu(-Ap) on ACT (else DVE tensor_scalar)
    mse_on_act=True,    # d^2 sum on ACT Square (always)
    ttr_acj=True,       # fused sliced TTR for Acj+sAcj (else TT + sliced ts)
    tree_stages=2,      # pairwise TT-min stages before the 3D min reduce
    ends_on_pool=True,  # end-of-curve extracts on GPSIMD (else DVE)
    epi_on_pool=True,   # epilogue tensor_tensor ops on GPSIMD (else DVE)
    dma_acj_first=True,
)


def _geom(v):
    F = v["F"]
    J = F // L
    M = NSH // (P * F)
    NCOL = M * J         # 64 for any F
    MSE0 = 0
    APN0 = MSE0 + M
    P30 = APN0 + M
    LS0 = P30 + NCOL
    E10 = LS0 + NCOL
    E20 = E10 + NCOL
    ACCW = E20 + NCOL
    return F, J, M, NCOL, MSE0, APN0, P30, LS0, E10, E20, ACCW


def _build_kernel(reps=None, variant=None):
    """reps=None: normal single-pass kernel. reps=R: wrap the whole body in a
    runtime For_i loop executing it R times (for HW timing via slope)."""
    v = dict(VARIANT)
    if variant:
        v.update(variant)
    F, J, M, NCOL, MSE0, APN0, P30, LS0, E10, E20, ACCW = _geom(v)

    nc = bacc.Bacc("TRN2", target_bir_lowering=False, debug=False, num_devices=NCORES)
    big = {
        nm: nc.declare_dram_parameter(nm, [NSH], f16, isOutput=False)
        for nm in ("An", "Ar", "Ac", "Aj", "Ap")
    }
    wdev = nc.declare_dram_parameter("wdev", [P, NCOL], f32, isOutput=False)
    fitw = nc.declare_dram_parameter("fitw", [P, NCOL], f32, isOutput=False)
    acc = nc.declare_dram_parameter("acc", [P, ACCW], f32, isOutput=True)

    with ExitStack() as ctx:
        tc = ctx.enter_context(tile.TileContext(nc))
        inp = ctx.enter_context(tc.tile_pool(name="inp", bufs=v["inp_bufs"]))
        wrk = ctx.enter_context(tc.tile_pool(name="wrk", bufs=v["wrk_bufs"]))
        per = ctx.enter_context(tc.tile_pool(name="per", bufs=1))

        g = {}
        for nm, shp, dt in (
            ("accT", [P, ACCW], f32), ("mnB", [P, NCOL], f16),
            ("sAcj", [P, NCOL], f32), ("sAbs", [P, NCOL], f32),
            ("gint", [P, NCOL], f32), ("eAp", [P, NCOL], f32),
            ("eAj", [P, NCOL], f32), ("eAc", [P, NCOL], f32),
            ("wT", [P, NCOL], f32), ("fT", [P, NCOL], f32),
            ("t1", [P, NCOL], f32), ("t2", [P, NCOL], f32),
            ("r1", [P, NCOL], f32), ("r2", [P, NCOL], f32),
            ("b8", [P, 1], f32),
        ):
            g[nm] = per.tile(shp, dt, tag=nm, name=nm)
        nc.vector.memset(g["b8"], 8.0)
        nc.sync.dma_start(out=g["wT"], in_=wdev[:])
        nc.sync.dma_start(out=g["fT"], in_=fitw[:])

        def body():
            _trace_body(nc, tc, big, acc, inp, wrk, g, v)

        if reps is None:
            body()
        else:
            with tc.For_i(0, reps, 1):
                body()

    nc.compile()
    return nc


def _trace_body(nc, tc, big, acc, inp, wrk, g, v):
    OP = mybir.AluOpType
    AF = mybir.ActivationFunctionType
    AX = mybir.AxisListType
    F, J, M, NCOL, MSE0, APN0, P30, LS0, E10, E20, ACCW = _geom(v)
    accT = g["accT"]

    for m in range(M):
        t = {}
        dma_order = ("Ac", "Aj", "Ap", "An", "Ar") if v["dma_acj_first"] \
            else ("An", "Ar", "Ac", "Aj", "Ap")
        for nm in dma_order:
            t[nm] = inp.tile([P, F], f16, tag=nm, name=f"in_{nm}_{m}")
            src = big[nm][:].rearrange("(m p f) -> m p f", m=M, p=P, f=F)[m]
            nc.sync.dma_start(out=t[nm], in_=src)

        cols = slice(m * J, (m + 1) * J)

        # --- GPSIMD: An-Ar subtract + end-of-curve extracts ---
        d = wrk.tile([P, F], f16, tag="d")
        d_eng = nc.gpsimd if v["d_on_pool"] else nc.vector
        d_eng.tensor_tensor(out=d, in0=t["An"], in1=t["Ar"], op=OP.subtract)
        ends_eng = nc.gpsimd if v["ends_on_pool"] else nc.vector
        for nm, dst in (("Ap", g["eAp"]), ("Aj", g["eAj"]), ("Ac", g["eAc"])):
            ends = t[nm].rearrange("p (j l) -> p j l", l=L)[:, :, L - 1 : L]
            ends_eng.tensor_copy(out=dst[:, cols], in_=ends)

        # --- ACT: global accumulations ---
        junk1 = wrk.tile([P, F], f16, tag="junk1")
        nc.scalar.activation(
            out=junk1, in_=d, func=AF.Square,
            accum_out=accT[:, MSE0 + m : MSE0 + m + 1],
        )
        apn_dst = accT[:, APN0 + m : APN0 + m + 1]
        if v["apn_on_act"]:
            junk2 = wrk.tile([P, F], f16, tag="junk2")
            nc.scalar.activation(
                out=junk2, in_=t["Ap"], func=AF.Relu, scale=-1.0,
                accum_out=apn_dst,
            )
        else:
            # accum = sum(min(Ap, 0)) = -sum(relu(-Ap)); negated on host.
            junk2 = wrk.tile([P, F], f16, tag="junk2")
            nc.vector.tensor_scalar(
                out=junk2, in0=t["Ap"], scalar1=0.0, scalar2=None,
                op0=OP.min, op1=OP.add, accum_out=apn_dst,
            )

        # --- DVE: Acj = Ac - Aj; per-curve sAcj via sliced ts-acc (4x) ---
        Acj = wrk.tile([P, F], f16, tag="Acj")
        nc.vector.tensor_tensor(out=Acj, in0=t["Ac"], in1=t["Aj"],
                                op=OP.subtract)
        junk3 = wrk.tile([P, L], f16, tag="junk3")
        for j in range(J):
            c = m * J + j
            sl = slice(j * L, (j + 1) * L)
            nc.vector.tensor_scalar(
                out=junk3, in0=Acj[:, sl], scalar1=1.0, scalar2=None,
                op0=OP.mult, op1=OP.add,
                accum_out=g["sAcj"][:, c : c + 1],
            )

        # --- ACT: A = |Acj| ; DVE: per-curve sAbs via sliced ts-acc ---
        A = wrk.tile([P, F], f16, tag="A")
        nc.scalar.activation(out=A, in_=Acj, func=AF.Abs)
        junk4 = wrk.tile([P, L], f16, tag="junk4")
        for j in range(J):
            c = m * J + j
            sl = slice(j * L, (j + 1) * L)
            nc.vector.tensor_scalar(
                out=junk4, in0=A[:, sl], scalar1=1.0, scalar2=None,
                op0=OP.mult, op1=OP.add,
                accum_out=g["sAbs"][:, c : c + 1],
            )

        # --- DVE: per-curve min via pairwise TT-min tree + 3D reduce ---
        A3 = A.rearrange("p (j l) -> p j l", l=L)
        stages = v["tree_stages"]
        cur3, ln = A3, L
        for s in range(stages):
            half = ln // 2
            Ts = wrk.tile([P, J * half], f16, tag=f"T{s}")
            Ts3 = Ts.rearrange("p (j h) -> p j h", h=half)
            nc.vector.tensor_tensor(
                out=Ts3, in0=cur3[:, :, 0:half], in1=cur3[:, :, half:ln],
                op=OP.min,
            )
            cur3, ln = Ts3, half
        nc.vector.tensor_reduce(out=g["mnB"][:, cols], in_=cur3, axis=AX.X,
                                op=OP.min)

        # --- DVE: G = 1.1*Aj - Ap; gint = sum (A==mn)*G per curve ---
        G = wrk.tile([P, F], f16, tag="G")
        nc.vector.scalar_tensor_tensor(
            out=G, in0=t["Aj"], scalar=1.1, in1=t["Ap"],
            op0=OP.mult, op1=OP.subtract,
        )
        junkD = wrk.tile([P, L], f16, tag="junkD")
        for j in range(J):
            c = m * J + j
            sl = slice(j * L, (j + 1) * L)
            nc.vector.scalar_tensor_tensor(
                out=junkD, in0=A[:, sl], scalar=g["mnB"][:, c : c + 1],
                in1=G[:, sl], op0=OP.is_equal, op1=OP.mult,
                accum_out=g["gint"][:, c : c + 1],
            )

    # --- epilogue on [128, NCOL] column blocks ---
    AF = mybir.ActivationFunctionType
    OP = mybir.AluOpType
    epi = nc.gpsimd if v["epi_on_pool"] else nc.vector
    t1, t2, r1, r2 = g["t1"], g["t2"], g["r1"], g["r2"]
    sAbs, sAcj, b8 = g["sAbs"], g["sAcj"], g["b8"]
    # ls penalty: relu(8-ls_Aj)+relu(8-ls_Ac), ls_* = (sAbs -+ sAcj)/2
    epi.tensor_tensor(out=t1, in0=sAbs, in1=sAcj, op=OP.add)
    nc.scalar.activation(out=r1, in_=t1, func=AF.Relu, scale=-0.5, bias=b8)
    epi.tensor_tensor(out=t2, in0=sAbs, in1=sAcj, op=OP.subtract)
    nc.scalar.activation(out=r2, in_=t2, func=AF.Relu, scale=-0.5, bias=b8)
    epi.tensor_tensor(out=t1, in0=r1, in1=r2, op=OP.add)
    epi.tensor_tensor(out=accT[:, LS0 : LS0 + NCOL], in0=t1, in1=g["wT"],
                      op=OP.mult)
    # crossover penalty: 3*relu(gint) == relu(3*gint)
    nc.scalar.activation(out=accT[:, P30 : P30 + NCOL], in_=g["gint"],
                         func=AF.Relu, scale=3.0)
    # end-of-curve penalties
    epi.tensor_tensor(out=t2, in0=g["eAp"], in1=g["eAj"], op=OP.subtract)
    nc.scalar.activation(out=r1, in_=t2, func=AF.Relu)
    epi.tensor_tensor(out=accT[:, E10 : E10 + NCOL], in0=r1, in1=g["fT"],
                      op=OP.mult)
    epi.tensor_tensor(out=t2, in0=g["eAj"], in1=g["eAc"], op=OP.subtract)
    nc.scalar.activation(out=accT[:, E20 : E20 + NCOL], in_=t2, func=AF.Relu)

    nc.sync.dma_start(out=acc[:], in_=accT)


_NC_CACHE = {}
LAST_RESULTS = None


def _get_nc(reps=None, variant=None):
    key = (reps, tuple(sorted((variant or {}).items())))
    if key not in _NC_CACHE:
        _NC_CACHE[key] = _build_kernel(reps, variant)
    return _NC_CACHE[key]


def _curve_layout(x_per_curve: np.ndarray, v=None) -> np.ndarray:
    """Map a per-curve [S] array for one core into the device [P, NCOL] layout:
    dev[p, m*J + j] corresponds to curve m*(P*J) + p*J + j."""
    F, J, M, NCOL = _geom(v or VARIANT)[:4]
    return np.ascontiguousarray(
        x_per_curve.reshape(M, P, J).transpose(1, 0, 2).reshape(P, NCOL)
    )


def prep_in_maps(An_o, Ac_o, Aj_o, Ap_o, A_r, Ci, mask_lightresp, v=None):
    w_full = (mask_lightresp == 0).astype(np.float32)        # [C]
    Ci_end = np.ascontiguousarray(Ci[L - 1 :: L])            # [C]
    fit_full = ((Ci_end > FIT_AP_CI).astype(np.float32) * w_full)  # [C]

    h = lambda x: np.ascontiguousarray(x, dtype=np.float16)
    in_maps = []
    for k in range(NCORES):
        cur = slice(k * S, (k + 1) * S)
        el = slice(k * NSH, (k + 1) * NSH)
        in_maps.append({
            "An": h(An_o[el]),
            "Ar": h(A_r[el]),
            "Ac": h(Ac_o[el]),
            "Aj": h(Aj_o[el]),
            "Ap": h(Ap_o[el]),
            "wdev": _curve_layout(w_full[cur], v),
            "fitw": _curve_layout(fit_full[cur], v),
        })
    return in_maps


def kernel(An_o, Ac_o, Aj_o, Ap_o, A_r, Ci, Vcmax25, Jmax25, Rd25,
           dHa_Vcmax, dHa_Jmax, dHa_TPU, Topt_Vcmax, Topt_Jmax, Topt_TPU,
           mask_lightresp):
    An_o, Ac_o, Aj_o, Ap_o, A_r, Ci = (
        np.asarray(x) for x in (An_o, Ac_o, Aj_o, Ap_o, A_r, Ci))
    (Vcmax25, Jmax25, Rd25, dHa_Vcmax, dHa_Jmax, dHa_TPU,
     Topt_Vcmax, Topt_Jmax, Topt_TPU, mask_lightresp) = (
        np.asarray(x) for x in (Vcmax25, Jmax25, Rd25, dHa_Vcmax, dHa_Jmax,
                                dHa_TPU, Topt_Vcmax, Topt_Jmax, Topt_TPU,
                                mask_lightresp))
    v = dict(VARIANT)
    F, J, M, NCOL, MSE0, APN0, P30, LS0, E10, E20, ACCW = _geom(v)
    nc = _get_nc()
    in_maps = prep_in_maps(An_o, Ac_o, Aj_o, Ap_o, A_r, Ci, mask_lightresp, v)

    try:
        res = run_bass_kernel_spmd(
            nc, in_maps, core_ids=list(range(NCORES)),
            trace=bool(int(os.environ.get("KERNEL_TRACE", "0"))),
        )
    except ModuleNotFoundError:
        os.environ["BASS_NEVER_TRACE"] = "1"
        res = run_bass_kernel_spmd(nc, in_maps, core_ids=list(range(NCORES)))
    global LAST_RESULTS
    LAST_RESULTS = res
    blocks = [r["acc"].astype(np.float64) for r in res.results]

    mse = sum(b[:, MSE0 : MSE0 + M].sum() for b in blocks)
    apn = sum(b[:, APN0 : APN0 + M].sum() for b in blocks)
    p3 = sum(b[:, P30 : P30 + NCOL].sum() for b in blocks)
    ls = sum(b[:, LS0 : LS0 + NCOL].sum() for b in blocks)
    e1 = sum(b[:, E10 : E10 + NCOL].sum() for b in blocks)
    e2 = sum(b[:, E20 : E20 + NCOL].sum() for b in blocks)
    if not v["apn_on_act"]:
        apn = -apn

    # host-side terms (tiny inputs only)
    w = (mask_lightresp == 0).astype(np.float64)
    x = Jmax25.astype(np.float64)
    y = Vcmax25.astype(np.float64)
    nw = w.sum()
    if nw > 0:
        my = (w * y).sum() / nw
        mx = (w * x).sum() / nw
        vy = (y - my) * w
        vx = (x - mx) * w
        denom = np.sqrt((vx * vx).sum()) * np.sqrt((vy * vy).sum())
        cost = (vx * vy).sum() / denom if denom != 0.0 else np.nan
    else:
        cost = np.nan
    if np.isnan(cost):
        cost = 0.0
    cost = min(cost, TARGET_R)

    relu = lambda z: np.maximum(z, 0.0)
    loss = mse * 10.0 / N
    loss += TARGET_R - cost
    loss += relu(-Rd25.astype(np.float64)).sum()
    loss += relu(-dHa_Vcmax.astype(np.float64)).sum() * 10.0
    loss += relu(-dHa_Jmax.astype(np.float64)).sum()
    loss += relu(-dHa_TPU.astype(np.float64)).sum()
    loss += relu(KELVIN - Topt_Vcmax.astype(np.float64)).sum()
    loss += relu(KELVIN - Topt_Jmax.astype(np.float64)).sum()
    loss += relu(KELVIN - Topt_TPU.astype(np.float64)).sum()
    loss += apn
    loss += e1 * 0.15
    loss += e2
    loss += p3
    loss += ls

    return np.asarray(loss, dtype=np.float32)
